# Initial kernel scaffold
#
"""Trainium2 Bass kernel for the NLNN (non-local neural network) block.

Reference semantics (per batch b, with X = x[b] as [1024, 2304] and N = 48*48):
    T   = w1 @ X            [512, 2304]
    PHI = w2 @ X            [512, 2304]
    G   = w3 @ X            [512, 2304]
    T'  = reshape(T,  [2304, 512])   (raw row-major memory reinterpretation)
    G'  = reshape(G,  [2304, 512])
    A   = softmax(T' @ PHI, axis=-1) [2304, 2304]
    Y   = A @ G'            [2304, 512]
    Yr  = reshape(Y, [512, 2304])
    out = X + w4 @ Yr + b4  [1024, 2304]

Sharding: pure data parallelism — batch B=8 mapped 1:1 onto 8 NeuronCores.

On-chip strategy (per core):
  - theta/phi convs and the logits matmul run in bf16 (precision-critical:
    exp amplifies logit error on this very peaked softmax).
  - g, exp(att), y and w4 are quantized to fp8-e4m3 so the Y matmul and
    the final conv run in DoubleRow mode (2 fp8 weights/PE cell, ~1.45x).
  - exp(att^T) is cast to fp8 with an exact per-column scale 128/colmax:
    DVE reduces the 18 m-tiles (swapped-AP tensor_reduce), GpSimd
    all-reduces across partitions, DVE takes the reciprocal and casts.
    The per-column scale cancels exactly in the softmax ratio since the
    ones-column denominator is scaled identically.
  - Softmax denominator comes for free: a ones-column is prepended to G'
    so the Y matmul accumulates sum_m ae[m, n] in PSUM column 0.
  - The awkward 4.5-ratio reshapes (T->T', G->G', Y->Yr) are realized by
    HBM round trips with natural access patterns; T' additionally gets the
    DMA xbar transpose. y/Yr round-trips through four per-row-block fp8
    tiles so each Yr read only waits on the strips that feed it.
  - b4 is folded into the residual x host-side, so the epilogue is a
    single DVE (psum * 2^-13 + x_in) op; the 2^13 is the fp8 scale
    product of y (x16) and w4 (x512).
"""

import numpy as np
import ml_dtypes

import concourse.bass as bass
import concourse.bacc as bacc
import concourse.mybir as mybir
import concourse.tile as tile
from concourse import bass_isa
from concourse.bass_utils import run_bass_kernel_spmd

F32 = mybir.dt.float32
BF16 = mybir.dt.bfloat16
F8 = mybir.dt.float8e4
AF = mybir.ActivationFunctionType
ALU = mybir.AluOpType
PM = mybir.MatmulPerfMode

C_IN = 1024
C_MID = 512
H = W = 48
N = H * W  # 2304
B = 8
NCORES = 8
KT = C_IN // 128   # 8  k tiles over input channels
MT = C_MID // 128  # 4  tiles over mid channels
NT = N // 128      # 18 tiles over spatial dim
# free-dim chunks of <=512 (one fp32 PSUM bank)
NCHUNKS = [(i, min(i + 512, N)) for i in range(0, N, 512)]
NROWS = N // MT    # 576 y rows per Yr row-tile
GW = 528           # padded gaug row width (16-aligned for DoubleRow steps)


def _emit(nc, tc, t_in, t_out):
    x_d = t_in["x"]

    with (
        tc.tile_pool(name="mega", bufs=1) as mega,
        tc.tile_pool(name="psum", bufs=8, space="PSUM") as psp,
        tc.tile_pool(name="dram", bufs=1, space="DRAM") as dramp,
        tc.tile_pool(name="small", bufs=4) as smallp,
    ):
        # ---- long-lived tiles (slots are re-tagged across phases) ----
        phi = mega.tile([128, MT, N], BF16, tag="phi")
        ttT = mega.tile([128, MT, N], BF16, tag="ttT")       # T'^T, [c, n]
        gaug = mega.tile([128, NT, 513], BF16, tag="gaug")   # [ones | G'*16]
        w4s8 = mega.tile([128, MT, C_IN], F8, tag="w4s")     # 512*w4, fp8
        bsml = mega.tile([128, 3 * MT], F32, tag="bsml")     # b1|b2|16*b3

        # flat HBM intermediates implementing the raw reshapes
        t_dram = dramp.tile([C_MID * N], BF16, tag="t_dram")
        g_dram = dramp.tile([C_MID * N], BF16, tag="g_dram")
        y_dram = [dramp.tile([NROWS * C_MID], F8, tag=f"y_dram{rt}",
                             name=f"y_dram{rt}")
                  for rt in range(MT)]
        t_w = t_dram[:].rearrange("(t p m) -> p t m", p=128, m=N)
        t_r = t_dram[:].rearrange("(n c) -> n c", c=C_MID)  # T' view [2304, 512]
        g_w = g_dram[:].rearrange("(t p m) -> p t m", p=128, m=N)
        g_r = g_dram[:].rearrange("(t p c) -> p t c", p=128, c=C_MID)  # G' tiles
        y_w = [yd[:].rearrange("(n c) -> n c", c=C_MID) for yd in y_dram]
        y_r = [yd[:].rearrange("(p m) -> p m", p=128) for yd in y_dram]

        # head loads: w1 k-slices interleaved with xb chunk-0 k-slices so the
        # first matmul can start after ~256KB instead of ~3.5MB.
        w1s = mega.tile([128, KT, C_MID], BF16, tag="w1s")
        xb = mega.tile([128, KT * N], BF16, tag="xmem")

        def xbr(ci, k):
            n0, n1 = NCHUNKS[ci]
            return xb[:, KT * n0 + k * (n1 - n0):KT * n0 + (k + 1) * (n1 - n0)]

        def load_xb(ci):
            n0, n1 = NCHUNKS[ci]
            mid = KT * n0 + (KT * (n1 - n0)) // 2
            nc.sync.dma_start(xb[:, KT * n0:mid], t_in["xb"][:, KT * n0:mid])
            nc.sync.dma_start(xb[:, mid:KT * n1], t_in["xb"][:, mid:KT * n1])

        w1v = t_in["w1t"][:].rearrange("p (t c) -> p t c", c=C_MID)
        for k in range(0, KT, 2):
            nc.sync.dma_start(w1s[:, k:k + 2, :], w1v[:, k:k + 2, :])
            nc.sync.dma_start(xb[:, 512 * k:512 * (k + 2)],
                              t_in["xb"][:, 512 * k:512 * (k + 2)])
        nc.sync.dma_start(bsml[:], t_in["bpack"][:])
        load_xb(1)
        load_xb(2)
        load_xb(3)
        load_xb(4)
        w2s = mega.tile([128, KT, C_MID], BF16, tag="w2s")
        nc.sync.dma_start(w2s[:], t_in["w2t"][:].rearrange("p (t c) -> p t c", c=C_MID))
        w3s = mega.tile([128, KT, C_MID], BF16, tag="w3s")
        nc.sync.dma_start(w3s[:], t_in["w3t"][:].rearrange("p (t c) -> p t c", c=C_MID))

        def conv(ws, boff, dest_sb, scale):
            """dest = scale*(w.T @ xb + bias); k-outer so chunk 0 can start
            on its first k-slices as they arrive."""
            for ci, (n0, n1) in enumerate(NCHUNKS):
                pss = []
                for mb in range(MT):
                    pss.append(psp.tile([128, n1 - n0], F32, tag="ps", name="ps"))
                for k in range(KT):
                    for mb in range(MT):
                        nc.tensor.matmul(
                            pss[mb][:],
                            lhsT=ws[:, k, mb * 128:(mb + 1) * 128],
                            rhs=xbr(ci, k),
                            start=(k == 0),
                            stop=(k == KT - 1),
                        )
                for mb in range(MT):
                    nc.scalar.activation(dest_sb[:, mb, n0:n1], pss[mb][:],
                                         AF.Identity, scale=scale,
                                         bias=bsml[:, boff * MT + mb:boff * MT + mb + 1])

        # theta conv first: its HBM round trip overlaps phi/g convs
        tstg = mega.tile([128, MT, N], BF16, tag="ae", bufs=2, name="tstg")
        conv(w1s, 0, tstg, 1.0)
        nc.sync.dma_start(t_w, tstg[:])
        # T'^T via xbar transpose reads of the flat T buffer
        for ct in range(MT):
            nc.sync.dma_start(
                ttT[:, ct, :], t_r[:, ct * 128:(ct + 1) * 128], transpose=True
            )
        conv(w2s, 1, phi, 1.0)
        nc.vector.memset(gaug[:, :, 0:1], 1.0)
        # g conv emits 16*g (bias pre-scaled host-side) so the y normalizer
        # directly yields the fp8 y scale
        gstg = mega.tile([128, MT, N], BF16, tag="ae", bufs=2, name="gstg")
        conv(w3s, 2, gstg, 16.0)
        nc.sync.dma_start(g_w[:, 0:2, :], gstg[:, 0:2, :])
        nc.sync.dma_start(gaug[:, 0:9, 1:513], g_r[:, 0:9, :])
        nc.sync.dma_start(g_w[:, 2:4, :], gstg[:, 2:4, :])
        nc.sync.dma_start(gaug[:, 9:18, 1:513], g_r[:, 9:18, :])

        # phase-E constants
        nc.sync.dma_start(w4s8[:], t_in["w4t"][:].rearrange("p (t c) -> p t c", c=C_IN))

        # fp32 x (with b4 pre-added host-side) for the residual
        NXF = 8
        xf = mega.tile([128, NXF, N], BF16, tag="xmem")

        # Yr row-tile PAIRS (DoubleRow moving operands), in dead conv-w slots
        yr8 = [mega.tile([128, 2, N], F8, tag=("w1s", "w2s")[i], name="yr8_t")
               for i in range(2)]

        def write_y8(ng, y_t):
            r0 = ng * 128
            rt = r0 // NROWS
            split = (rt + 1) * NROWS - r0  # rows in this tile (<=128)
            if split >= 128:
                nc.sync.dma_start(y_w[rt][r0 - rt * NROWS:r0 - rt * NROWS + 128, :],
                                  y_t[:])
            else:
                nc.sync.dma_start(y_w[rt][r0 - rt * NROWS:, :], y_t[0:split, :])
                nc.sync.dma_start(y_w[rt + 1][0:128 - split, :], y_t[split:128, :])

        # ---- attention + Y, strip by strip over n ----
        for si, (n0, n1) in enumerate(NCHUNKS):
            wn = n1 - n0
            ae = mega.tile([128, NT, wn], BF16, tag="ae", bufs=2, name="ae")
            for ci in range(si * 2, min(si * 2 + 2, NXF)) if si < 4 else range(0):
                nc.sync.dma_start(xf[:, ci, :], x_d[ci * 128:(ci + 1) * 128, :])
            for mb in range(NT):
                ps = psp.tile([128, wn], F32, tag="ps")
                for ct in range(MT):
                    nc.tensor.matmul(
                        ps[:],
                        lhsT=phi[:, ct, mb * 128:(mb + 1) * 128],
                        rhs=ttT[:, ct, n0:n1],
                        start=(ct == 0),
                        stop=(ct == MT - 1),
                    )
                nc.scalar.activation(ae[:, mb, :], ps[:], AF.Exp)
            for nbl in range(wn // 128):
                psA = psp.tile([128, 257], F32, tag="ps")
                psB = psp.tile([128, 256], F32, tag="ps")
                for mt in range(NT):
                    lhs = ae[:, mt, nbl * 128:(nbl + 1) * 128]
                    nc.tensor.matmul(psA[:], lhsT=lhs, rhs=gaug[:, mt, 0:257],
                                     start=(mt == 0), stop=(mt == NT - 1))
                    nc.tensor.matmul(psB[:], lhsT=lhs, rhs=gaug[:, mt, 257:513],
                                     start=(mt == 0), stop=(mt == NT - 1))
                rcp = smallp.tile([128, 1], F32, tag="rcp")
                nc.vector.reciprocal(rcp[:], psA[:, 0:1])
                # g carries a x16 scale, so psA/den is directly 16*y -> fp8
                y_t = smallp.tile([128, C_MID], F8, tag="yt")
                nc.vector.tensor_scalar_mul(y_t[:, 0:256], psA[:, 1:257], rcp[:])
                nc.vector.tensor_scalar_mul(y_t[:, 256:512], psB[:], rcp[:])
                write_y8(n0 // 128 + nbl, y_t)
                if si == 4 and nbl == 0:
                    # partitions 64-98 cover y rows <= 2173 (written by
                    # strips 3 + 4/nbl0); partition 99 crosses into nbl1
                    nc.sync.dma_start(yr8[1][64:99, 1, :], y_r[3][64:99, :])
                elif si == 4 and nbl == 1:
                    nc.sync.dma_start(yr8[1][99:128, 1, :], y_r[3][99:128, :])
            # stagger Yr pair-tile quarter loads right after the strip that
            # completes their source rows
            if si == 1:
                nc.sync.dma_start(yr8[0][:, 0, :], y_r[0])
            elif si == 2:
                nc.sync.dma_start(yr8[0][:, 1, :], y_r[1])
            elif si == 3:
                nc.sync.dma_start(yr8[1][:, 0, :], y_r[2])
                # first half of rt3 (y rows 1728-2015) is complete after
                # strip 3 as well
                nc.sync.dma_start(yr8[1][0:64, 1, :], y_r[3][0:64, :])
            # (si == 4 tail reads are emitted inside the nbl loop below)

        # ---- final conv + residual: out = x_in + w4 @ Yr  (b4 in x_in) ----
        for cb in range(KT):
            xcb = xf[:, cb, :]
            out_t = mega.tile([128, N], BF16, tag=("ttT", "phi", "gaug")[cb % 3],
                              name="out_t")
            for ci, (n0, n1) in enumerate(NCHUNKS):
                ps = psp.tile([128, n1 - n0], F32, tag="ps", name="ps")
                nc.tensor.matmul(ps[:], lhsT=w4s8[:, 0:2, cb * 128:(cb + 1) * 128],
                                 rhs=yr8[0][:, :, n0:n1],
                                 start=True, stop=False, perf_mode=PM.DoubleRow)
                nc.tensor.matmul(ps[:], lhsT=w4s8[:, 2:4, cb * 128:(cb + 1) * 128],
                                 rhs=yr8[1][:, :, n0:n1],
                                 start=False, stop=True, perf_mode=PM.DoubleRow)
                nc.vector.scalar_tensor_tensor(
                    out_t[:, n0:n1], ps[:], 2.0 ** -13,
                    xcb[:, n0:n1], op0=ALU.mult, op1=ALU.add,
                )
                if cb < KT - 1:
                    if n1 == 1024:
                        nc.sync.dma_start(t_out[cb * 128:(cb + 1) * 128, 0:1024],
                                          out_t[:, 0:1024])
                else:
                    nc.sync.dma_start(t_out[cb * 128:(cb + 1) * 128, n0:n1],
                                      out_t[:, n0:n1])
            if cb < KT - 1:
                nc.sync.dma_start(t_out[cb * 128:(cb + 1) * 128, 1024:N],
                                  out_t[:, 1024:N])


def build_module():
    nc = bacc.Bacc("TRN2", target_bir_lowering=False, debug=False)
    t_in = {
        "x": nc.dram_tensor("x", [C_IN, N], BF16, kind="ExternalInput").ap(),
        "xb": nc.dram_tensor("xb", [128, KT * N], BF16, kind="ExternalInput").ap(),
        "w1t": nc.dram_tensor("w1t", [128, KT * C_MID], BF16, kind="ExternalInput").ap(),
        "w2t": nc.dram_tensor("w2t", [128, KT * C_MID], BF16, kind="ExternalInput").ap(),
        "w3t": nc.dram_tensor("w3t", [128, KT * C_MID], BF16, kind="ExternalInput").ap(),
        "w4t": nc.dram_tensor("w4t", [128, MT * C_IN], F8, kind="ExternalInput").ap(),
        "bpack": nc.dram_tensor("bpack", [128, 3 * MT], F32, kind="ExternalInput").ap(),
    }
    t_out = nc.dram_tensor("out", [C_IN, N], BF16, kind="ExternalOutput").ap()
    with tile.TileContext(nc) as tc:
        _emit(nc, tc, t_in, t_out)
    nc.compile()
    return nc


_NC = None


def _get_nc():
    global _NC
    if _NC is None:
        _NC = build_module()
    return _NC


def _ptile(a):
    """[T*128, C] -> [128, T*C] with the 128-partition dim outermost."""
    t = a.shape[0] // 128
    return np.ascontiguousarray(
        a.reshape(t, 128, a.shape[1]).transpose(1, 0, 2).reshape(128, -1)
    )


def make_in_maps(x, w1, b1, w2, b2, w3, b3, w4, b4):
    bf = ml_dtypes.bfloat16
    f8 = ml_dtypes.float8_e4m3
    bpack = np.stack(
        [np.asarray(b1, np.float32).reshape(MT, 128),
         np.asarray(b2, np.float32).reshape(MT, 128),
         16.0 * np.asarray(b3, np.float32).reshape(MT, 128)],
        axis=0,
    ).transpose(2, 0, 1).reshape(128, 3 * MT)
    shared = {
        "w1t": _ptile(np.asarray(w1, np.float32).T).astype(bf),
        "w2t": _ptile(np.asarray(w2, np.float32).T).astype(bf),
        "w3t": _ptile(np.asarray(w3, np.float32).T).astype(bf),
        "w4t": np.clip(_ptile(np.asarray(w4, np.float32).T) * 512.0,
                       -240.0, 240.0).astype(f8),
        "bpack": np.ascontiguousarray(bpack),
    }
    x = np.asarray(x, np.float32)
    xpb = x.reshape(B, C_IN, N) + np.asarray(b4, np.float32)[None, :, None]
    maps = []
    for i in range(B):
        xi = np.ascontiguousarray(x[i].reshape(C_IN, N))
        x8 = xi.reshape(KT, 128, N)
        xbt = np.concatenate(
            [x8[:, :, n0:n1].transpose(1, 0, 2).reshape(128, -1)
             for (n0, n1) in NCHUNKS], axis=1)
        maps.append({"x": np.ascontiguousarray(xpb[i]).astype(bf),
                     "xb": np.ascontiguousarray(xbt).astype(bf), **shared})
    return maps


def _run(in_maps, **kw):
    return run_bass_kernel_spmd(_get_nc(), in_maps, list(range(NCORES)), **kw)


def kernel(x, w1, b1, w2, b2, w3, b3, w4, b4):
    res = _run(make_in_maps(x, w1, b1, w2, b2, w3, b3, w4, b4))
    out = np.stack([np.asarray(res.results[i]["out"]) for i in range(B)])
    return out.reshape(B, C_IN, H, W).astype(np.float32)



# revision 1
# speedup vs baseline: 1.0332x; 1.0332x over previous
"""Trainium2 Bass kernel for the NLNN (non-local neural network) block.

Reference semantics (per batch b, with X = x[b] as [1024, 2304] and N = 48*48):
    T   = w1 @ X            [512, 2304]
    PHI = w2 @ X            [512, 2304]
    G   = w3 @ X            [512, 2304]
    T'  = reshape(T,  [2304, 512])   (raw row-major memory reinterpretation)
    G'  = reshape(G,  [2304, 512])
    A   = softmax(T' @ PHI, axis=-1) [2304, 2304]
    Y   = A @ G'            [2304, 512]
    Yr  = reshape(Y, [512, 2304])
    out = X + w4 @ Yr + b4  [1024, 2304]

Sharding: pure data parallelism — batch B=8 mapped 1:1 onto 8 NeuronCores.

On-chip strategy (per core):
  - theta/phi convs and the logits matmul run in bf16 (precision-critical:
    exp amplifies logit error on this very peaked softmax).
  - g, exp(att), y and w4 are quantized to fp8-e4m3 so the Y matmul and
    the final conv run in DoubleRow mode (2 fp8 weights/PE cell, ~1.45x).
  - exp(att^T) is cast to fp8 with an exact per-column scale 128/colmax:
    DVE reduces the 18 m-tiles (swapped-AP tensor_reduce), GpSimd
    all-reduces across partitions, DVE takes the reciprocal and casts.
    The per-column scale cancels exactly in the softmax ratio since the
    ones-column denominator is scaled identically.
  - Softmax denominator comes for free: a ones-column is prepended to G'
    so the Y matmul accumulates sum_m ae[m, n] in PSUM column 0.
  - The awkward 4.5-ratio reshapes (T->T', G->G', Y->Yr) are realized by
    HBM round trips with natural access patterns; T' additionally gets the
    DMA xbar transpose. y/Yr round-trips through four per-row-block fp8
    tiles so each Yr read only waits on the strips that feed it.
  - b4 is folded into the residual x host-side, so the epilogue is a
    single DVE (psum * 2^-13 + x_in) op; the 2^13 is the fp8 scale
    product of y (x16) and w4 (x512).
"""

import numpy as np
import ml_dtypes

import concourse.bass as bass
import concourse.bacc as bacc
import concourse.mybir as mybir
import concourse.tile as tile
from concourse import bass_isa
from concourse.bass_utils import run_bass_kernel_spmd

F32 = mybir.dt.float32
BF16 = mybir.dt.bfloat16
F8 = mybir.dt.float8e4
AF = mybir.ActivationFunctionType
ALU = mybir.AluOpType
PM = mybir.MatmulPerfMode

C_IN = 1024
C_MID = 512
H = W = 48
N = H * W  # 2304
B = 8
NCORES = 8
KT = C_IN // 128   # 8  k tiles over input channels
MT = C_MID // 128  # 4  tiles over mid channels
NT = N // 128      # 18 tiles over spatial dim
# free-dim chunks of <=512 (one fp32 PSUM bank)
NCHUNKS = [(i, min(i + 512, N)) for i in range(0, N, 512)]
NROWS = N // MT    # 576 y rows per Yr row-tile
GW = 528           # padded gaug row width (16-aligned for DoubleRow steps)


def _emit(nc, tc, t_in, t_out):
    x_d = t_in["x"]

    with (
        tc.tile_pool(name="mega", bufs=1) as mega,
        tc.tile_pool(name="psum", bufs=8, space="PSUM") as psp,
        tc.tile_pool(name="dram", bufs=1, space="DRAM") as dramp,
        tc.tile_pool(name="small", bufs=4) as smallp,
    ):
        # ---- long-lived tiles (slots are re-tagged across phases) ----
        phi = mega.tile([128, MT, N], BF16, tag="phi")
        ttT = mega.tile([128, MT, N], BF16, tag="ttT")       # T'^T, [c, n]
        gaug = mega.tile([128, NT, 513], BF16, tag="gaug")   # [ones | G'*16]
        w4s8 = mega.tile([128, MT, C_IN], F8, tag="w4s")     # 512*w4, fp8
        bsml = mega.tile([128, 3 * MT], F32, tag="bsml")     # b1|b2|16*b3

        # flat HBM intermediates implementing the raw reshapes
        t_dram = dramp.tile([C_MID * N], BF16, tag="t_dram")
        g_dram = dramp.tile([C_MID * N], BF16, tag="g_dram")
        y_dram = [dramp.tile([NROWS * C_MID], F8, tag=f"y_dram{rt}",
                             name=f"y_dram{rt}")
                  for rt in range(MT)]
        t_w = t_dram[:].rearrange("(t p m) -> p t m", p=128, m=N)
        t_r = t_dram[:].rearrange("(n c) -> n c", c=C_MID)  # T' view [2304, 512]
        g_w = g_dram[:].rearrange("(t p m) -> p t m", p=128, m=N)
        g_r = g_dram[:].rearrange("(t p c) -> p t c", p=128, c=C_MID)  # G' tiles
        y_w = [yd[:].rearrange("(n c) -> n c", c=C_MID) for yd in y_dram]
        y_r = [yd[:].rearrange("(p m) -> p m", p=128) for yd in y_dram]

        # head loads: w1 k-slices interleaved with xb chunk-0 k-slices so the
        # first matmul can start after ~256KB instead of ~3.5MB.
        w1s = mega.tile([128, KT, C_MID], BF16, tag="w1s")
        xb = mega.tile([128, KT * N], BF16, tag="xmem")

        def xbr(ci, k):
            n0, n1 = NCHUNKS[ci]
            return xb[:, KT * n0 + k * (n1 - n0):KT * n0 + (k + 1) * (n1 - n0)]

        def load_xb(ci):
            n0, n1 = NCHUNKS[ci]
            mid = KT * n0 + (KT * (n1 - n0)) // 2
            nc.sync.dma_start(xb[:, KT * n0:mid], t_in["xb"][:, KT * n0:mid])
            nc.sync.dma_start(xb[:, mid:KT * n1], t_in["xb"][:, mid:KT * n1])

        w1v = t_in["w1t"][:].rearrange("p (t c) -> p t c", c=C_MID)
        for k in range(0, KT, 2):
            nc.sync.dma_start(w1s[:, k:k + 2, :], w1v[:, k:k + 2, :])
            nc.sync.dma_start(xb[:, 512 * k:512 * (k + 2)],
                              t_in["xb"][:, 512 * k:512 * (k + 2)])
        nc.sync.dma_start(bsml[:], t_in["bpack"][:])
        load_xb(1)
        load_xb(2)
        load_xb(3)
        load_xb(4)
        w2s = mega.tile([128, KT, C_MID], BF16, tag="w2s")
        nc.sync.dma_start(w2s[:], t_in["w2t"][:].rearrange("p (t c) -> p t c", c=C_MID))
        w3s = mega.tile([128, KT, C_MID], BF16, tag="w3s")
        nc.sync.dma_start(w3s[:], t_in["w3t"][:].rearrange("p (t c) -> p t c", c=C_MID))

        def conv(ws, boff, dest_sb, scale):
            """dest = scale*(w.T @ xb + bias); k-outer so chunk 0 can start
            on its first k-slices as they arrive."""
            for ci, (n0, n1) in enumerate(NCHUNKS):
                pss = []
                for mb in range(MT):
                    pss.append(psp.tile([128, n1 - n0], F32, tag="ps", name="ps"))
                for k in range(KT):
                    for mb in range(MT):
                        nc.tensor.matmul(
                            pss[mb][:],
                            lhsT=ws[:, k, mb * 128:(mb + 1) * 128],
                            rhs=xbr(ci, k),
                            start=(k == 0),
                            stop=(k == KT - 1),
                        )
                for mb in range(MT):
                    nc.scalar.activation(dest_sb[:, mb, n0:n1], pss[mb][:],
                                         AF.Identity, scale=scale,
                                         bias=bsml[:, boff * MT + mb:boff * MT + mb + 1])

        # theta conv first: its HBM round trip overlaps phi/g convs
        tstg = mega.tile([128, MT, N], BF16, tag="ae", bufs=2, name="tstg")
        conv(w1s, 0, tstg, 1.0)
        nc.sync.dma_start(t_w, tstg[:])
        # T'^T via xbar transpose reads of the flat T buffer
        for ct in range(MT):
            nc.sync.dma_start(
                ttT[:, ct, :], t_r[:, ct * 128:(ct + 1) * 128], transpose=True
            )
        conv(w2s, 1, phi, 1.0)
        nc.vector.memset(gaug[:, :, 0:1], 1.0)
        # g conv emits 16*g (bias pre-scaled host-side) so the y normalizer
        # directly yields the fp8 y scale
        gstg = mega.tile([128, MT, N], BF16, tag="ae", bufs=2, name="gstg")
        conv(w3s, 2, gstg, 16.0)
        nc.sync.dma_start(g_w[:, 0:2, :], gstg[:, 0:2, :])
        nc.sync.dma_start(gaug[:, 0:9, 1:513], g_r[:, 0:9, :])
        nc.sync.dma_start(g_w[:, 2:4, :], gstg[:, 2:4, :])
        nc.sync.dma_start(gaug[:, 9:18, 1:513], g_r[:, 9:18, :])

        # phase-E constants
        nc.sync.dma_start(w4s8[:], t_in["w4t"][:].rearrange("p (t c) -> p t c", c=C_IN))

        # fp32 x (with b4 pre-added host-side) for the residual
        NXF = 8
        xf = mega.tile([128, NXF, N], BF16, tag="xmem")

        # Yr row-tile PAIRS (DoubleRow moving operands), in dead conv-w slots
        yr8 = [mega.tile([128, 2, N], F8, tag=("w1s", "w2s")[i], name="yr8_t")
               for i in range(2)]

        def write_y8(ng, y_t):
            r0 = ng * 128
            rt = r0 // NROWS
            split = (rt + 1) * NROWS - r0  # rows in this tile (<=128)
            if split >= 128:
                nc.sync.dma_start(y_w[rt][r0 - rt * NROWS:r0 - rt * NROWS + 128, :],
                                  y_t[:])
            else:
                nc.sync.dma_start(y_w[rt][r0 - rt * NROWS:, :], y_t[0:split, :])
                nc.sync.dma_start(y_w[rt + 1][0:128 - split, :], y_t[split:128, :])

        # ---- attention + Y, strip by strip over n ----
        for si, (n0, n1) in enumerate(NCHUNKS):
            wn = n1 - n0
            ae = mega.tile([128, NT, wn], BF16, tag="ae", bufs=2, name="ae")
            for ci in range(si * 2, min(si * 2 + 2, NXF)) if si < 4 else range(0):
                nc.sync.dma_start(xf[:, ci, :], x_d[ci * 128:(ci + 1) * 128, :])
            for mb in range(NT):
                ps = psp.tile([128, wn], F32, tag="ps")
                for ct in range(MT):
                    nc.tensor.matmul(
                        ps[:],
                        lhsT=phi[:, ct, mb * 128:(mb + 1) * 128],
                        rhs=ttT[:, ct, n0:n1],
                        start=(ct == 0),
                        stop=(ct == MT - 1),
                    )
                nc.scalar.activation(ae[:, mb, :], ps[:], AF.Exp)
            for nbl in range(wn // 128):
                psA = psp.tile([128, 257], F32, tag="ps")
                psB = psp.tile([128, 256], F32, tag="ps")
                for mt in range(NT):
                    lhs = ae[:, mt, nbl * 128:(nbl + 1) * 128]
                    nc.tensor.matmul(psA[:], lhsT=lhs, rhs=gaug[:, mt, 0:257],
                                     start=(mt == 0), stop=(mt == NT - 1))
                    nc.tensor.matmul(psB[:], lhsT=lhs, rhs=gaug[:, mt, 257:513],
                                     start=(mt == 0), stop=(mt == NT - 1))
                rcp = smallp.tile([128, 1], F32, tag="rcp")
                nc.vector.reciprocal(rcp[:], psA[:, 0:1])
                # g carries a x16 scale, so psA/den is directly 16*y -> fp8
                y_t = smallp.tile([128, C_MID], F8, tag="yt")
                nc.vector.tensor_scalar_mul(y_t[:, 0:256], psA[:, 1:257], rcp[:])
                nc.vector.tensor_scalar_mul(y_t[:, 256:512], psB[:], rcp[:])
                write_y8(n0 // 128 + nbl, y_t)
                if si == 4 and nbl == 0:
                    # partitions 64-98 cover y rows <= 2173 (written by
                    # strips 3 + 4/nbl0); partition 99 crosses into nbl1
                    nc.sync.dma_start(yr8[1][64:99, 1, :], y_r[3][64:99, :])
                elif si == 4 and nbl == 1:
                    nc.sync.dma_start(yr8[1][99:128, 1, :], y_r[3][99:128, :])
            # stagger Yr pair-tile quarter loads right after the strip that
            # completes their source rows
            if si == 1:
                nc.sync.dma_start(yr8[0][:, 0, :], y_r[0])
            elif si == 2:
                nc.sync.dma_start(yr8[0][:, 1, :], y_r[1])
            elif si == 3:
                nc.sync.dma_start(yr8[1][:, 0, :], y_r[2])
                # first half of rt3 (y rows 1728-2015) is complete after
                # strip 3 as well
                nc.sync.dma_start(yr8[1][0:64, 1, :], y_r[3][0:64, :])
            # (si == 4 tail reads are emitted inside the nbl loop below)

        # ---- final conv + residual: out = x_in + w4 @ Yr  (b4 in x_in) ----
        for cb in range(KT):
            xcb = xf[:, cb, :]
            out_t = mega.tile([128, N], BF16, tag=("ttT", "phi", "gaug")[cb % 3],
                              name="out_t")
            for ci, (n0, n1) in enumerate(NCHUNKS):
                ps = psp.tile([128, n1 - n0], F32, tag="ps", name="ps")
                nc.tensor.matmul(ps[:], lhsT=w4s8[:, 0:2, cb * 128:(cb + 1) * 128],
                                 rhs=yr8[0][:, :, n0:n1],
                                 start=True, stop=False, perf_mode=PM.DoubleRow)
                nc.tensor.matmul(ps[:], lhsT=w4s8[:, 2:4, cb * 128:(cb + 1) * 128],
                                 rhs=yr8[1][:, :, n0:n1],
                                 start=False, stop=True, perf_mode=PM.DoubleRow)
                nc.vector.scalar_tensor_tensor(
                    out_t[:, n0:n1], ps[:], 2.0 ** -13,
                    xcb[:, n0:n1], op0=ALU.mult, op1=ALU.add,
                )
                if cb < KT - 1:
                    if n1 == 1024:
                        nc.sync.dma_start(t_out[cb * 128:(cb + 1) * 128, 0:1024],
                                          out_t[:, 0:1024])
                else:
                    nc.sync.dma_start(t_out[cb * 128:(cb + 1) * 128, n0:n1],
                                      out_t[:, n0:n1])
            if cb < KT - 1:
                nc.sync.dma_start(t_out[cb * 128:(cb + 1) * 128, 1024:N],
                                  out_t[:, 1024:N])


def build_module():
    nc = bacc.Bacc("TRN2", target_bir_lowering=False, debug=False)
    t_in = {
        "x": nc.dram_tensor("x", [C_IN, N], BF16, kind="ExternalInput").ap(),
        "xb": nc.dram_tensor("xb", [128, KT * N], BF16, kind="ExternalInput").ap(),
        "w1t": nc.dram_tensor("w1t", [128, KT * C_MID], BF16, kind="ExternalInput").ap(),
        "w2t": nc.dram_tensor("w2t", [128, KT * C_MID], BF16, kind="ExternalInput").ap(),
        "w3t": nc.dram_tensor("w3t", [128, KT * C_MID], BF16, kind="ExternalInput").ap(),
        "w4t": nc.dram_tensor("w4t", [128, MT * C_IN], F8, kind="ExternalInput").ap(),
        "bpack": nc.dram_tensor("bpack", [128, 3 * MT], F32, kind="ExternalInput").ap(),
    }
    t_out = nc.dram_tensor("out", [C_IN, N], BF16, kind="ExternalOutput").ap()
    with tile.TileContext(nc) as tc:
        _emit(nc, tc, t_in, t_out)
    nc.compile()
    return nc


_NC = None


def _get_nc():
    global _NC
    if _NC is None:
        _NC = build_module()
    return _NC


def _ptile(a):
    """[T*128, C] -> [128, T*C] with the 128-partition dim outermost."""
    t = a.shape[0] // 128
    return np.ascontiguousarray(
        a.reshape(t, 128, a.shape[1]).transpose(1, 0, 2).reshape(128, -1)
    )


def make_in_maps(x, w1, b1, w2, b2, w3, b3, w4, b4):
    bf = ml_dtypes.bfloat16
    f8 = ml_dtypes.float8_e4m3
    bpack = np.stack(
        [np.asarray(b1, np.float32).reshape(MT, 128),
         np.asarray(b2, np.float32).reshape(MT, 128),
         16.0 * np.asarray(b3, np.float32).reshape(MT, 128)],
        axis=0,
    ).transpose(2, 0, 1).reshape(128, 3 * MT)
    shared = {
        "w1t": _ptile(np.asarray(w1, np.float32).T).astype(bf),
        "w2t": _ptile(np.asarray(w2, np.float32).T).astype(bf),
        "w3t": _ptile(np.asarray(w3, np.float32).T).astype(bf),
        "w4t": np.clip(_ptile(np.asarray(w4, np.float32).T) * 512.0,
                       -240.0, 240.0).astype(f8),
        "bpack": np.ascontiguousarray(bpack),
    }
    x = np.asarray(x, np.float32)
    xpb = x.reshape(B, C_IN, N) + np.asarray(b4, np.float32)[None, :, None]
    maps = []
    for i in range(B):
        xi = np.ascontiguousarray(x[i].reshape(C_IN, N))
        x8 = xi.reshape(KT, 128, N)
        xbt = np.concatenate(
            [x8[:, :, n0:n1].transpose(1, 0, 2).reshape(128, -1)
             for (n0, n1) in NCHUNKS], axis=1)
        maps.append({"x": np.ascontiguousarray(xpb[i]).astype(bf),
                     "xb": np.ascontiguousarray(xbt).astype(bf), **shared})
    return maps


def _run(in_maps, **kw):
    return run_bass_kernel_spmd(_get_nc(), in_maps, list(range(NCORES)), **kw)


def kernel(x, w1, b1, w2, b2, w3, b3, w4, b4):
    res = _run(make_in_maps(x, w1, b1, w2, b2, w3, b3, w4, b4))
    out = np.stack([np.asarray(res.results[i]["out"]) for i in range(B)])
    return out.reshape(B, C_IN, H, W).astype(np.float32)

